# revision 13
# baseline (speedup 1.0000x reference)
"""Cross-attention kernel for 8 Trainium2 NeuronCores.

Sharding: core c => batch b = c//4, head-group g = c%4 (3 of 12 heads, 192 dims).
Each core projects q/k/v for its heads, does softmax attention, and computes a
partial output projection (row-split Wo); host sums the 4 partials per batch.

The kernel is paced by the ScalarE exp stream (one [128,1024] exp per
(head, key-chunk), ~1.08us each); everything else is scheduled to hide under
it:

  - mask compaction on host: only mask==1 key/value positions are shipped,
    zero-padded to a multiple of 128; pad rows have zero v and a zero
    ones-column so they contribute 0 to numerator and denominator.
  - per-head j-loops, software-pipelined: scores(j) -> exp(j) -> attnV(j-1).
  - scores run ROW-PAIRED on the PE (tile_position (0,0)/(64,0)): nf=0:512
    streams on array rows 0-63 while nf=512:1024 streams on rows 64-127
    concurrently (2x).  k/q for heads 0/1 are duplicated into the upper
    partition half by SBUF->SBUF DMA; head 2 runs serial scores (fits).
  - attnV keeps the ones-column (M=65) giving the denominator Z for free.
  - inputs stream over BOTH HWDGE rings in first-use order as contiguous
    per-block tensors (fat descriptors); PE warmup matmuls engage the HAM
    clock gate before the real stream.
  - 1/Z: exact DVE reciprocal for heads 0/1 emitted LATE in the next head's
    loop (its only consumer is the tail Wo), broadcast + normalize muls on
    GpSimd, so nothing urgent ever queues behind the 6.5us op.
  - Wo runs entirely at the tail: per query tile, all three heads accumulate
    into one PSUM pair (row-paired halves), one DVE cast, one out-DMA -- no
    mid-kernel staging traffic.  The tail reciprocal is split in halves so
    the first query tiles start ~3us earlier.
  - f16 output partials (host accumulates in f32 and adds bo).
"""

import numpy as np

import concourse.bass as bass
import concourse.mybir as mybir
import concourse.tile as tile
from concourse import bacc
from concourse.bass_utils import run_bass_kernel_spmd

H = 12
D = 768
HD = 64
NQ = 1024
HL = 3            # heads per core
DC = 6            # 768 / 128 contraction chunks
SCALE = HD ** -0.5

f16 = mybir.dt.float16
f32 = mybir.dt.float32

LAST_EXEC_NS = None
LAST_RESULT = None

_programs = {}


def _blocks(SP):
    """Column blocks of the key/value panels, in DMA order."""
    cuts = list(range(0, SP, 512)) + [SP]
    cuts = sorted(set(min(c, SP) for c in cuts))
    return list(zip(cuts[:-1], cuts[1:]))


def _build(SP: int):
    NCH = SP // 128
    SPA = min(1024, SP)
    EXPF = mybir.ActivationFunctionType.Exp
    BLKS = _blocks(SP)

    nc = bacc.Bacc("TRN2", target_bir_lowering=False, debug=False, num_devices=8)

    pk = nc.dram_tensor("pk", [128, DC, 576], f16, kind="ExternalInput")
    msk = nc.dram_tensor("msk", [128, NCH], f16, kind="ExternalInput")
    qT = nc.dram_tensor("qT", [128, DC, NQ], f16, kind="ExternalInput")
    kts = [nc.dram_tensor(f"kT{i}", [128, DC, b - a], f16, kind="ExternalInput")
           for i, (a, b) in enumerate(BLKS)]
    vts = [nc.dram_tensor(f"vT{i}", [128, DC, b - a], f16, kind="ExternalInput")
           for i, (a, b) in enumerate(BLKS)]
    wo = nc.dram_tensor("wo", [128, HL, D], f16, kind="ExternalInput")
    out = nc.dram_tensor("out", [NQ, D], f16, kind="ExternalOutput")

    with tile.TileContext(nc) as tc:
        with (
            tc.tile_pool(name="const", bufs=1) as cp,
            tc.tile_pool(name="nrm", bufs=2) as np_,
            tc.tile_pool(name="expp", bufs=4) as ep,
            tc.tile_pool(name="obuf", bufs=2) as op_,
            tc.tile_pool(name="pssc", bufs=2, space="PSUM") as pssc,
            tc.tile_pool(name="psat", bufs=1, space="PSUM") as psat,
            tc.tile_pool(name="psf", bufs=2, space="PSUM") as psf,
        ):
            # ---------------- persistent tiles
            pk_in = cp.tile([128, DC, 576], f16)
            msk_in = cp.tile([128, NCH], f16)
            qT_in = cp.tile([128, DC, NQ], f16)
            kT_in = cp.tile([128, DC, SP], f16)
            vT_in = cp.tile([128, DC, SP], f16)
            wo_in = cp.tile([128, HL, D], f16)
            qh = cp.tile([128, HL, NQ], f16)      # per-head q (lo; hi dup'd)
            kh = cp.tile([128, HL, SP], f16)      # per-head k (lo; hi dup'd)
            vaug = cp.tile([128, HL * NCH * 65], f16)
            vaug_r = vaug[:].rearrange("p (h j e) -> p h j e", h=HL, j=NCH)
            a_all = cp.tile([128, HL, NQ], f16)   # normalized attn out, dup'd

            # ---------------- DMA, in first-use order over both HWDGE rings
            # sync ring: weights + k path (feeds the score chain)
            # scalar ring: q + v path
            kT_r = kT_in[:]
            vT_r = vT_in[:]
            nc.sync.dma_start(pk_in[:], pk.ap())
            nc.scalar.dma_start(msk_in[:], msk.ap())
            nc.scalar.dma_start(qT_in[:], qT.ap())
            for i, (a, b) in enumerate(BLKS):
                nc.sync.dma_start(kT_r[:, :, a:b], kts[i].ap())
                nc.scalar.dma_start(vT_r[:, :, a:b], vts[i].ap())
            nc.sync.dma_start(wo_in[:], wo.ap())

            # ---------------- PE warmup: junk matmuls on pk to engage HAM
            for i in range(20):
                ps = psf.tile([128, 512], f32, tag="f")
                nc.tensor.matmul(ps[:], pk_in[:, 0, 0:128], pk_in[:, 1, 0:512],
                                 start=True, stop=True)

            # mask column of vaug (dep: msk only)
            nc.vector.tensor_copy(
                vaug_r[:, :, :, 64],
                msk_in[:].rearrange("p (u j) -> p u j", u=1)
                .broadcast_to([128, HL, NCH]),
            )

            # ---------------- projection helpers
            def proj01(wcol, src_r, dst, nf, wf):
                """Project heads 0+1 (M=128); evac lo halves only."""
                ps = psf.tile([128, 512], f32, tag="f")
                for d in range(DC):
                    nc.tensor.matmul(
                        ps[:, 0:wf], pk_in[:, d, wcol:wcol + 128],
                        src_r[:, d, nf:nf + wf],
                        start=(d == 0), stop=(d == DC - 1),
                    )
                nc.vector.tensor_copy(dst[0:64, 0, nf:nf + wf], ps[0:64, 0:wf])
                nc.vector.tensor_copy(dst[0:64, 1, nf:nf + wf], ps[64:128, 0:wf])

            def dup01(dst, nf, wf):
                """Duplicate heads-0/1 lo data into the upper partition half
                (SBUF->SBUF DMA on the gpsimd SWDGE ring)."""
                nc.gpsimd.dma_start(dst[64:128, 0:2, nf:nf + wf],
                                    dst[0:64, 0:2, nf:nf + wf])

            def proj2_pair(jobs):
                """Head-2 q/k projections as col-tile pairs (M=64)."""
                for i in range(0, len(jobs), 2):
                    pair = jobs[i:i + 2]
                    pst = [psf.tile([128, 512], f32, tag="f", name=f"p2_{i}_{t}")
                           for t in range(len(pair))]
                    for d in range(DC):
                        for t, (wcol, src_r, dst, nf, wf) in enumerate(pair):
                            nc.tensor.matmul(
                                pst[t][64 * t:64 * t + 64, 0:wf],
                                pk_in[:, d, wcol:wcol + 64],
                                src_r[:, d, nf:nf + wf],
                                start=(d == 0), stop=(d == DC - 1),
                            )
                    for t, (wcol, src_r, dst, nf, wf) in enumerate(pair):
                        nc.vector.tensor_copy(
                            dst[0:64, 2, nf:nf + wf],
                            pst[t][64 * t:64 * t + 64, 0:wf])

            def proj_v(j):
                ps = psf.tile([128, 192], f32, tag="f")
                for d in range(DC):
                    nc.tensor.matmul(
                        ps[:], vT_r[:, d, j * 128:(j + 1) * 128],
                        pk_in[:, d, 384:576],
                        start=(d == 0), stop=(d == DC - 1),
                    )
                nc.vector.tensor_copy(
                    vaug_r[:, :, j, 0:64],
                    ps[:].rearrange("p (h e) -> p h e", h=HL),
                )

            # ---------------- prologue projections (DMA-paced)
            for nf in range(0, NQ, 512):
                proj01(0, qT_in, qh, nf, 512)              # q heads 0,1
            dup01(qh, 0, NQ)
            for nf in range(0, SPA, 512):
                proj01(192, kT_r, kh, nf, 512)             # k heads 0,1 (cols a)
                dup01(kh, nf, 512)
            for j in range(4):                             # first v chunks
                proj_v(j)

            # per-(head,iteration) scheduled work.  sched[h][j] = list of
            # thunks emitted right after that iteration's attnV.
            sched = [[[] for _ in range(NCH)] for _ in range(HL)]

            # h0 fillers: v chunks + late-k projections, deadline-ordered
            h0q = [(lambda j=j: proj_v(j)) for j in range(4, NCH)]
            kbs = [(nf, min(512, SP - nf)) for nf in range(SPA, SP, 512)]

            def kb_job(nf, wf):
                proj01(192, kT_r, kh, nf, wf)
                dup01(kh, nf, wf)

            h0q2 = []
            vi = 0
            while vi < len(h0q) or kbs:
                h0q2.extend(h0q[vi:vi + 2])
                vi += 2
                if kbs:
                    nf, wf = kbs.pop(0)
                    h0q2.append(lambda nf=nf, wf=wf: kb_job(nf, wf))
            # head-2 q/k ragged col pairs after the k/v stream
            p2jobs = [(128, qT_in, qh, nf, 512) for nf in range(0, NQ, 512)]
            k2jobs = [(320, kT_r, kh, nf, min(512, SP - nf))
                      for nf in range(0, SP, 512)]
            mixed = [p2jobs[0], k2jobs[0], p2jobs[1], k2jobs[1]] + k2jobs[2:]
            h0q2 += [(lambda i=i: proj2_pair(mixed[i:i + 2]))
                     for i in range(0, len(mixed), 2)]
            for j in range(NCH):
                if j < len(h0q2):
                    sched[0][j].append(h0q2[j])
            for i, t in enumerate(h0q2[NCH:]):      # overflow -> h1 early
                sched[1][i].append(t)

            # normalize chains for heads 0/1: emitted late in the NEXT head's
            # loop (only consumer is the tail Wo), all off the critical path.
            ats = [None] * HL
            rzbs = [None] * HL

            def norm_step(h, step):
                if step == 0:
                    rz = np_.tile([1, NQ], f32, tag="rz", name=f"rz{h}")
                    rb = np_.tile([64, NQ], f32, tag="rzb", name=f"rzb{h}")
                    rzbs[h] = (rz, rb)
                    nc.vector.reciprocal(rz[:], ats[h][64:65, :])
                elif step == 1:
                    rz, rb = rzbs[h]
                    nc.gpsimd.partition_broadcast(rb[:], rz[:])
                elif step == 2:
                    rz, rb = rzbs[h]
                    nc.gpsimd.tensor_mul(a_all[0:64, h, :], ats[h][0:64, :], rb[:])
                else:
                    rz, rb = rzbs[h]
                    nc.gpsimd.tensor_mul(a_all[64:128, h, :], ats[h][0:64, :],
                                         rb[:])

            for h in (0, 1):
                for step in range(4):
                    sched[h + 1][min(11 + step, NCH - 1)].append(
                        lambda h=h, step=step: norm_step(h, step))

            # ---------------- attention loops (ScalarE-paced)
            for h in range(HL):
                at = psat.tile([65, NQ], f32, tag="at", name=f"at{h}")
                prev = None
                for j in range(NCH):
                    sc = pssc.tile([128, NQ], f32, tag="sc", name=f"sc{h}_{j}")
                    if h < 2:
                        nc.tensor.matmul(
                            sc[:, 0:512], kh[0:64, h, j * 128:(j + 1) * 128],
                            qh[0:64, h, 0:512], start=True, stop=True,
                        )
                        nc.tensor.matmul(
                            sc[:, 512:1024], kh[64:128, h, j * 128:(j + 1) * 128],
                            qh[64:128, h, 512:1024], start=True, stop=True,
                        )
                    else:
                        for nf in range(0, NQ, 512):
                            nc.tensor.matmul(
                                sc[:, nf:nf + 512],
                                kh[0:64, h, j * 128:(j + 1) * 128],
                                qh[0:64, h, nf:nf + 512], start=True, stop=True,
                            )
                    ex = ep.tile([128, NQ], f16, tag="ex", name=f"ex{h}_{j}")
                    nc.scalar.activation(ex[:], sc[:], EXPF, scale=SCALE)
                    if prev is not None:
                        pj, pex = prev
                        for nf in range(0, NQ, 512):
                            nc.tensor.matmul(
                                at[:, nf:nf + 512],
                                vaug_r[:, h, pj, :], pex[:, nf:nf + 512],
                                start=(pj == 0), stop=False,
                            )
                    prev = (j, ex)
                    for t in sched[h][j]:
                        t()
                pj, pex = prev
                for nf in range(0, NQ, 512):
                    nc.tensor.matmul(
                        at[:, nf:nf + 512],
                        vaug_r[:, h, pj, :], pex[:, nf:nf + 512],
                        start=(pj == 0), stop=True,
                    )
                # evacuate accumulator so the single PSUM slot recycles fast
                ats_h = np_.tile([65, NQ], f32, tag="ats", name=f"ats{h}")
                ats[h] = ats_h
                nc.vector.tensor_copy(ats_h[:], at[:])

            # ---------------- tail: h2 normalize + all-head Wo + out DMA
            HD2 = D // 2
            rz2 = np_.tile([1, NQ], f32, tag="rz", name="rz2")
            rzb2 = np_.tile([64, NQ], f32, tag="rzb", name="rzb2")
            for half in range(2):
                hf = slice(half * 512, half * 512 + 512)
                nc.vector.reciprocal(rz2[:, hf], ats[2][64:65, hf])
                nc.gpsimd.partition_broadcast(rzb2[:, hf], rz2[:, hf])
                nc.gpsimd.tensor_mul(a_all[0:64, 2, hf], ats[2][0:64, hf],
                                     rzb2[:, hf])
                nc.gpsimd.tensor_mul(a_all[64:128, 2, hf], ats[2][0:64, hf],
                                     rzb2[:, hf])
                for nt in range(half * 4, half * 4 + 4):
                    pa = psf.tile([128, HD2], f32, tag="f", name=f"tpa{nt}")
                    pb = psf.tile([128, HD2], f32, tag="f", name=f"tpb{nt}")
                    for h in range(HL):
                        nc.tensor.matmul(
                            pa[:], a_all[0:64, h, nt * 128:(nt + 1) * 128],
                            wo_in[0:64, h, 0:HD2],
                            start=(h == 0), stop=(h == HL - 1),
                        )
                        nc.tensor.matmul(
                            pb[:], a_all[64:128, h, nt * 128:(nt + 1) * 128],
                            wo_in[64:128, h, HD2:D],
                            start=(h == 0), stop=(h == HL - 1),
                        )
                    obf = op_.tile([128, D], f16, tag="obf", name=f"obf{nt}")
                    nc.vector.tensor_copy(obf[:, 0:HD2], pa[:])
                    nc.vector.tensor_copy(obf[:, HD2:D], pb[:])
                    nc.sync.dma_start(out[nt * 128:(nt + 1) * 128, :], obf[:])

    nc.compile()
    return nc


def _get_program(SP: int):
    if SP not in _programs:
        _programs[SP] = _build(SP)
    return _programs[SP]


def _rearr(x, dt=np.float16):
    """[768, n] -> [128, 6, n] d-chunk layout, contiguous."""
    return np.ascontiguousarray(
        x.reshape(DC, 128, -1).transpose(1, 0, 2).astype(dt))


def kernel(query, key, value, mask, Wq, Wk, Wv, Wo, bo):
    query = np.asarray(query, np.float32)
    key = np.asarray(key, np.float32)
    value = np.asarray(value, np.float32)
    mask = np.asarray(mask, np.float32)
    Wq = np.asarray(Wq, np.float32)
    Wk = np.asarray(Wk, np.float32)
    Wv = np.asarray(Wv, np.float32)
    Wo = np.asarray(Wo, np.float32)
    bo = np.asarray(bo, np.float32)

    B, N, _ = query.shape
    idxs = [np.nonzero(mask[b] > 0.5)[0] for b in range(B)]
    se_max = max(len(i) for i in idxs)
    SP = max(((se_max + 127) // 128) * 128, 128)
    NCH = SP // 128
    BLKS = _blocks(SP)
    nc = _get_program(SP)

    HWID = HL * HD
    in_maps = []
    for c in range(8):
        b, g = c // 4, c % 4
        hs = g * HWID
        idx = idxs[b]
        ne = len(idx)
        kTc = np.zeros((D, SP), np.float32)
        kTc[:, :ne] = key[b].T[:, idx]
        vTc = np.zeros((D, SP), np.float32)
        vTc[:, :ne] = value[b].T[:, idx]
        # msk[p, j] = 1 iff compacted key index j*128+p is a real key
        sidx = np.arange(SP).reshape(NCH, 128).T
        mvec = (sidx < ne).astype(np.float16)

        wq_r = _rearr(Wq[hs:hs + HWID, :].T)      # [128, 6, 192]
        wk_r = _rearr(Wk[hs:hs + HWID, :].T)
        wv_r = _rearr(Wv[hs:hs + HWID, :].T)
        pk = np.concatenate([wq_r, wk_r, wv_r], axis=2)  # [128, 6, 576]

        # wo planes: head h rows dup'd into both partition halves
        woc = Wo[:, hs:hs + HWID].T.astype(np.float16)   # [192, 768]
        wop = np.empty((128, HL, D), np.float16)
        for h in range(HL):
            wop[0:64, h, :] = woc[h * 64:(h + 1) * 64, :]
            wop[64:128, h, :] = woc[h * 64:(h + 1) * 64, :]

        kTr = _rearr(kTc)
        vTr = _rearr(vTc)
        m = {
            "pk": np.ascontiguousarray(pk),
            "msk": mvec,
            "qT": _rearr(query[b].T),
            "wo": wop,
        }
        for i, (a, bb) in enumerate(BLKS):
            m[f"kT{i}"] = np.ascontiguousarray(kTr[:, :, a:bb])
            m[f"vT{i}"] = np.ascontiguousarray(vTr[:, :, a:bb])
        in_maps.append(m)

    r = run_bass_kernel_spmd(nc, in_maps, list(range(8)))
    global LAST_EXEC_NS, LAST_RESULT
    LAST_EXEC_NS = r.exec_time_ns
    LAST_RESULT = r
    res = r.results
    out = np.zeros((B, N, D), np.float32)
    for b in range(B):
        acc = res[4 * b]["out"].astype(np.float32)
        for g in range(1, 4):
            acc += res[4 * b + g]["out"].astype(np.float32)
        out[b] = acc + bo
    return out


# revision 22
# speedup vs baseline: 1.2651x; 1.2651x over previous
"""Cross-attention kernel for 8 Trainium2 NeuronCores.

Sharding: core c => batch b = c//4, head-group g = c%4 (3 of 12 heads, 192 dims).
Each core projects q/k/v for its heads, does softmax attention, and computes a
partial output projection (row-split Wo); host sums the 4 partials per batch.

Key tricks:
  - mask compaction on host: only mask==1 key/value positions are shipped
    (~2048 of 4096), zero-padded to a multiple of 128. Padded rows have
    zeroed v and zeroed ones-column so they contribute 0 to both numerator
    and denominator => exact equivalence with the reference's -1e4 bias.
  - transposed layouts end to end (contraction dim on partitions): no
    on-device transposes.
  - softmax without max-subtraction (scores*scale ~ N(0,1): exp safe in
    fp32) and without dividing the SxN score matrix: a ones-column appended
    to v yields the denominator Z per output row; only the 64xN attention
    output is normalized.
  - fp16 operands for all matmuls (fp32 PSUM accumulate).
  - q/k/v projections are spread through the attention j-loops as PE filler
    so the PE stream stays dense (HAM clock gate at 8/8), with emission
    software-pipelined (scores j+1 issued before attn j).
  - Wo fillers in the h2 loop start at j=2 so the first h2 score matmuls
    (which depend on nothing new) run while the h0/h1 normalize chain
    completes, instead of the PE queue stalling on a0.
"""

import numpy as np

import concourse.bass as bass
import concourse.mybir as mybir
import concourse.tile as tile
from concourse import bacc
from concourse.bass_utils import run_bass_kernel_spmd

# Force every activation onto the one table set that holds both exp and ln,
# so the ScalarE never reloads tables mid-kernel (saves ~5us of stalls).
# Entries stay in place (empty sets never match) so act_func_set_id mapping
# is unchanged.
import concourse.bacc as _bacc_mod
from concourse.hw_specs import get_activation_tables as _orig_gat


def _patched_gat(arch):
    tabs = _orig_gat(arch)
    keep = "natural_log_exp_and_others"
    if keep not in tabs:
        return tabs
    return {name: (fns if name == keep else set())
            for name, fns in tabs.items()}


_bacc_mod.get_activation_tables = _patched_gat

H = 12
D = 768
HD = 64
SCALE = HD ** -0.5
NQ = 1024
HL = 3            # heads per core
HWID = HL * HD    # 192 head dims per core
DC = D // 128     # 6 contraction chunks

f16 = mybir.dt.float16
f32 = mybir.dt.float32

LAST_EXEC_NS = None
LAST_RESULT = None

_programs = {}


def _build(SP: int):
    NCH = SP // 128
    nc = bacc.Bacc("TRN2", target_bir_lowering=False, debug=False, num_devices=8)

    qT = nc.dram_tensor("qT", [D, NQ], f16, kind="ExternalInput")
    kT = nc.dram_tensor("kT", [D, SP], f16, kind="ExternalInput")
    vT = nc.dram_tensor("vT", [D, SP], f16, kind="ExternalInput")
    mv = nc.dram_tensor("mv", [SP], f16, kind="ExternalInput")
    wqT = nc.dram_tensor("wqT", [D, HWID], f16, kind="ExternalInput")
    wkT = nc.dram_tensor("wkT", [D, HWID], f16, kind="ExternalInput")
    wvT = nc.dram_tensor("wvT", [D, HWID], f16, kind="ExternalInput")
    woT = nc.dram_tensor("woT", [HWID, D], f16, kind="ExternalInput")
    out = nc.dram_tensor("out", [NQ, D], f16, kind="ExternalOutput")

    EXPF = mybir.ActivationFunctionType.Exp
    qT_r = qT.ap().rearrange("(c p) n -> p c n", p=128)
    kT_r = kT.ap().rearrange("(c p) n -> p c n", p=128)
    vT_r = vT.ap().rearrange("(c p) n -> p c n", p=128)

    with tile.TileContext(nc) as tc:
        with (
            tc.tile_pool(name="const", bufs=1) as cpool,
            tc.tile_pool(name="work", bufs=2) as wpool,
            tc.tile_pool(name="expp", bufs=14) as epool,
            tc.tile_pool(name="ps", bufs=2, space="PSUM") as pspool,
            tc.tile_pool(name="psa", bufs=2, space="PSUM") as psapool,
        ):
            # ---- input DMAs, chunked + ordered so compute starts early
            wq_in = cpool.tile([128, DC, HWID], f16)
            nc.sync.dma_start(wq_in[:], wqT.ap().rearrange("(c p) n -> p c n", p=128))
            wk_in = cpool.tile([128, DC, HWID], f16)
            nc.sync.dma_start(wk_in[:], wkT.ap().rearrange("(c p) n -> p c n", p=128))
            SPA = min(1024, SP)          # first column block of kT/vT
            qT_in = cpool.tile([128, DC, NQ], f16)
            for d in range(DC):
                nc.sync.dma_start(qT_in[:, d, :], qT_r[:, d, :])
            kT_in = cpool.tile([128, DC, SP], f16)
            for d in range(DC):
                nc.sync.dma_start(kT_in[:, d, 0:SPA], kT_r[:, d, 0:SPA])
            wv_in = cpool.tile([128, DC, HWID], f16)
            nc.sync.dma_start(wv_in[:], wvT.ap().rearrange("(c p) n -> p c n", p=128))
            vT_in = cpool.tile([128, DC, SP], f16)
            for d in range(DC):
                nc.sync.dma_start(vT_in[:, d, 0:SPA], vT_r[:, d, 0:SPA])
            if SP > SPA:
                for d in range(DC):
                    nc.sync.dma_start(kT_in[:, d, SPA:SP], kT_r[:, d, SPA:SP])
                for d in range(DC):
                    nc.sync.dma_start(vT_in[:, d, SPA:SP], vT_r[:, d, SPA:SP])
            wo_in = cpool.tile([128, 2, D], f16)
            nc.sync.dma_start(wo_in[:, 0, :], woT[0:128, :])
            nc.sync.dma_start(wo_in[0:64, 1, :], woT[128:HWID, :])
            msk = cpool.tile([128, NCH], f16)
            nc.sync.dma_start(msk[:], mv.ap().rearrange("(c p) -> p c", p=128))

            q0 = cpool.tile([128, NQ], f16)
            q1 = cpool.tile([64, NQ], f16)
            k0 = cpool.tile([128, SP], f16)
            k1 = cpool.tile([64, SP], f16)
            vaug = cpool.tile([128, HL * NCH * 65], f16)
            vaug_r = vaug[:].rearrange("p (h j e) -> p h j e", h=HL, j=NCH)
            a0 = cpool.tile([128, NQ], f16)
            a1 = cpool.tile([64, NQ], f16)

            def proj_qk(w_in, src, dst, mt, sg, sw):
                mw = 128 if mt == 0 else 64
                ps = pspool.tile([mw, sw], f32, tag="ps")
                for d in range(DC):
                    for nf in range(0, sw, 512):
                        wf = min(512, sw - nf)
                        nc.tensor.matmul(
                            ps[:, nf:nf + wf],
                            w_in[:, d, mt * 128:mt * 128 + mw],
                            src[:, d, sg + nf:sg + nf + wf],
                            start=(d == 0), stop=(d == DC - 1),
                        )
                nc.vector.tensor_copy(dst[:, sg:sg + sw], ps[:])

            def proj_v(j):
                ps = pspool.tile([128, HWID], f32, tag="ps")
                for d in range(DC):
                    nc.tensor.matmul(
                        ps[:], vT_in[:, d, j * 128:(j + 1) * 128], wv_in[:, d, :],
                        start=(d == 0), stop=(d == DC - 1),
                    )
                nc.vector.tensor_copy(
                    vaug_r[:, :, j, 0:64], ps[:].rearrange("p (h e) -> p h e", h=HL)
                )

            def wo_mms(po, nt, kk, start, stop):
                asrc, kw = ((a0, 128), (a1, 64))[kk]
                for nf in range(0, D, 512):
                    wf = min(512, D - nf)
                    nc.tensor.matmul(
                        po[:, nf:nf + wf],
                        asrc[:, nt * 128:(nt + 1) * 128],
                        wo_in[0:kw, kk, nf:nf + wf],
                        start=start, stop=stop,
                    )

            LNF = mybir.ActivationFunctionType.Ln

            def normalize(at, adst):
                # 1/Z = exp(-ln Z), on ScalarE (keeps the DVE queue clear;
                # DVE's iterative reciprocal on [1, N] costs ~6.5us)
                lz = wpool.tile([1, NQ], f32, tag="lz")
                nc.scalar.activation(lz[:], at[64:65, :], LNF)
                rz = wpool.tile([1, NQ], f32, tag="rz")
                nc.scalar.activation(rz[:], lz[:], EXPF, scale=-1.0)
                rzb = wpool.tile([64, NQ], f32, tag="rzb")
                nc.gpsimd.partition_broadcast(rzb[:], rz[:])
                nc.vector.tensor_mul(adst, at[0:64, :], rzb[:])

            # mask column of vaug (depends only on msk DMA)
            nc.vector.tensor_copy(
                vaug_r[:, :, :, 64],
                msk[:].rearrange("p (u j) -> p u j", u=1).broadcast_to([128, HL, NCH]),
            )

            # ---- prologue: all projections, dense PE stream (DMA-paced)
            for sg in range(0, NQ, 1024):
                proj_qk(wq_in, qT_in, q0, 0, sg, min(1024, NQ - sg))
            for sg in range(0, SPA, 1024):
                proj_qk(wk_in, kT_in, k0, 0, sg, min(1024, SPA - sg))
            for j in range(SPA // 128):
                proj_v(j)
            for sg in range(0, NQ, 1024):
                proj_qk(wq_in, qT_in, q1, 1, sg, min(1024, NQ - sg))
            for sg in range(SPA, SP, 1024):
                proj_qk(wk_in, kT_in, k0, 0, sg, min(1024, SP - sg))
            for j in range(SPA // 128, NCH):
                proj_v(j)
            for sg in range(0, SP, 1024):
                proj_qk(wk_in, kT_in, k1, 1, sg, min(1024, SP - sg))

            # ---- fused h0+h1 attention (scores row-packed: K=64 pair at
            # base partitions 0/64 runs concurrently in the PE array)
            at0 = psapool.tile([65, NQ], f32, tag="at")
            at1 = psapool.tile([65, NQ], f32, tag="at")
            prev = None
            for j in range(NCH):
                sc0 = pspool.tile([128, NQ], f32, tag="ps")
                sc1 = pspool.tile([128, NQ], f32, tag="ps")
                for nf in range(0, NQ, 512):
                    nc.tensor.matmul(
                        sc0[:, nf:nf + 512], k0[0:64, j * 128:(j + 1) * 128],
                        q0[0:64, nf:nf + 512], start=True, stop=True,
                    )
                    nc.tensor.matmul(
                        sc1[:, nf:nf + 512], k0[64:128, j * 128:(j + 1) * 128],
                        q0[64:128, nf:nf + 512], start=True, stop=True,
                    )
                ex0 = epool.tile([128, NQ], f16, tag="ex")
                nc.scalar.activation(ex0[:], sc0[:], EXPF, scale=SCALE)
                ex1 = epool.tile([128, NQ], f16, tag="ex")
                nc.scalar.activation(ex1[:], sc1[:], EXPF, scale=SCALE)
                if prev is not None:
                    pj, pex0, pex1 = prev
                    for nf in range(0, NQ, 512):
                        nc.tensor.matmul(
                            at0[:, nf:nf + 512],
                            vaug[:, (0 * NCH + pj) * 65:(0 * NCH + pj) * 65 + 65],
                            pex0[:, nf:nf + 512], start=(pj == 0), stop=False,
                        )
                        nc.tensor.matmul(
                            at1[:, nf:nf + 512],
                            vaug[:, (1 * NCH + pj) * 65:(1 * NCH + pj) * 65 + 65],
                            pex1[:, nf:nf + 512], start=(pj == 0), stop=False,
                        )
                prev = (j, ex0, ex1)
            pj, pex0, pex1 = prev
            for nf in range(0, NQ, 512):
                nc.tensor.matmul(
                    at0[:, nf:nf + 512],
                    vaug[:, (0 * NCH + pj) * 65:(0 * NCH + pj) * 65 + 65],
                    pex0[:, nf:nf + 512], start=(pj == 0), stop=True,
                )
                nc.tensor.matmul(
                    at1[:, nf:nf + 512],
                    vaug[:, (1 * NCH + pj) * 65:(1 * NCH + pj) * 65 + 65],
                    pex1[:, nf:nf + 512], start=(pj == 0), stop=True,
                )
            normalize(at0, a0[0:64, :])
            normalize(at1, a0[64:128, :])

            # ---- h2 attention, with Wo kk=0 accumulation as PE filler.
            # Fillers start at j=2: the first h2 score matmuls depend only on
            # k1/q1 (long ready), so the PE stays busy while the h0/h1
            # normalize chain (Scalar ln/exp -> GpSimd broadcast -> DVE mul)
            # produces a0; previously wo_mms(nt=0) sat at the queue head
            # waiting for a0 and the PE idled ~5us.
            at2 = psapool.tile([65, NQ], f32, tag="at")
            ob_a = cpool.tile([128, NQ // 128, D], f32)   # staged a0 @ WoT[0:128]
            prev = None
            for j in range(NCH):
                if j >= 2 and j % 2 == 0 and (j - 2) // 2 < NQ // 128:
                    nt = (j - 2) // 2
                    po = psapool.tile([128, D], f32, tag="at")
                    wo_mms(po, nt, 0, True, True)
                    nc.vector.tensor_copy(ob_a[:, nt, :], po[:])
                sc = pspool.tile([128, NQ], f32, tag="ps")
                for nf in range(0, NQ, 512):
                    nc.tensor.matmul(
                        sc[:, nf:nf + 512], k1[:, j * 128:(j + 1) * 128],
                        q1[:, nf:nf + 512], start=True, stop=True,
                    )
                ex = epool.tile([128, NQ], f16, tag="ex")
                nc.scalar.activation(ex[:], sc[:], EXPF, scale=SCALE)
                if prev is not None:
                    pj, pex = prev
                    for nf in range(0, NQ, 512):
                        nc.tensor.matmul(
                            at2[:, nf:nf + 512],
                            vaug[:, (2 * NCH + pj) * 65:(2 * NCH + pj) * 65 + 65],
                            pex[:, nf:nf + 512], start=(pj == 0), stop=False,
                        )
                prev = (j, ex)
            pj, pex = prev
            for nf in range(0, NQ, 512):
                nc.tensor.matmul(
                    at2[:, nf:nf + 512],
                    vaug[:, (2 * NCH + pj) * 65:(2 * NCH + pj) * 65 + 65],
                    pex[:, nf:nf + 512], start=(pj == 0), stop=True,
                )
            normalize(at2, a1[:, :])

            # ---- finish Wo: kk=1 into PSUM, add staged kk=0 part, DMA out
            for nt in range(NQ // 128):
                po = pspool.tile([128, D], f32, tag="ps")
                wo_mms(po, nt, 1, True, True)
                ob = wpool.tile([128, D], f16, tag="ob")
                with nc.allow_low_precision(reason="f16 partial output"):
                    nc.vector.tensor_add(ob[:], po[:], ob_a[:, nt, :])
                nc.sync.dma_start(out[nt * 128:(nt + 1) * 128, :], ob[:])
    nc.compile()
    return nc


def _get_program(SP: int):
    if SP not in _programs:
        _programs[SP] = _build(SP)
    return _programs[SP]


def kernel(query, key, value, mask, Wq, Wk, Wv, Wo, bo):
    query = np.asarray(query, np.float32)
    key = np.asarray(key, np.float32)
    value = np.asarray(value, np.float32)
    mask = np.asarray(mask, np.float32)
    Wq = np.asarray(Wq, np.float32)
    Wk = np.asarray(Wk, np.float32)
    Wv = np.asarray(Wv, np.float32)
    Wo = np.asarray(Wo, np.float32)
    bo = np.asarray(bo, np.float32)

    B, N, _ = query.shape
    idxs = [np.nonzero(mask[b] > 0.5)[0] for b in range(B)]
    se_max = max(len(i) for i in idxs)
    SP = max(((se_max + 127) // 128) * 128, 128)
    nc = _get_program(SP)

    in_maps = []
    for c in range(8):
        b, g = c // 4, c % 4
        hs = g * HWID
        idx = idxs[b]
        ne = len(idx)
        kTc = np.zeros((D, SP), np.float16)
        kTc[:, :ne] = key[b].T[:, idx].astype(np.float16)
        vTc = np.zeros((D, SP), np.float16)
        vTc[:, :ne] = value[b].T[:, idx].astype(np.float16)
        mvec = np.zeros((SP,), np.float16)
        mvec[:ne] = 1.0
        in_maps.append({
            "qT": np.ascontiguousarray(query[b].T.astype(np.float16)),
            "kT": kTc,
            "vT": vTc,
            "mv": mvec,
            "wqT": np.ascontiguousarray(Wq[hs:hs + HWID, :].T.astype(np.float16)),
            "wkT": np.ascontiguousarray(Wk[hs:hs + HWID, :].T.astype(np.float16)),
            "wvT": np.ascontiguousarray(Wv[hs:hs + HWID, :].T.astype(np.float16)),
            "woT": np.ascontiguousarray(Wo[:, hs:hs + HWID].T.astype(np.float16)),
        })

    r = run_bass_kernel_spmd(nc, in_maps, list(range(8)))
    global LAST_EXEC_NS, LAST_RESULT
    LAST_EXEC_NS = r.exec_time_ns
    LAST_RESULT = r
    res = r.results
    out = np.zeros((B, N, D), np.float32)
    for b in range(B):
        out[b] = (res[4 * b]["out"].astype(np.float32)
                  + res[4 * b + 1]["out"].astype(np.float32)
                  + res[4 * b + 2]["out"].astype(np.float32)
                  + res[4 * b + 3]["out"].astype(np.float32) + bo)
    return out

